# revision 1
# baseline (speedup 1.0000x reference)
"""Multi-head attention (strictly-upper-triangular mask variant) on 8 TRN2 cores.

Reference math (B=4, S=2048, D=512, H=8, A=64):
    q/k/v = per-head projections of query/key/value           [B,H,S,A]
    scores = q @ k^T / sqrt(A), masked where k <= q (lower triangle incl diag
    masked to -1e9 -> softmax attends strictly to FUTURE positions)
    out = concat_heads(softmax(scores) @ v) @ Wo + bo         [B,S,D]

Sharding: 8 cores = 4 batches x 2 interleaved q-tile sets.  Core c handles
batch b=c//2, q-tiles g = 2*i + (c%2) for i in 0..7 (128 rows each).  Every
core computes all 8 heads for its 1024 query rows; no collectives needed —
the host gather is a pure row-interleave concat.

Device dataflow (per core, all matmuls bf16 with fp32 PSUM accumulation):
    QT/KT/VT = Wx^T-stacked projections in [A,S] layout (head-pair stacked
    to 128 partitions), computed from host-pre-transposed x^T inputs.
    V (natural [k,a] layout) obtained by PE-transposing VT; stationary AV
    operand carries a 64-wide block of ones so the matmul itself replicates
    the softmax denominator across 64 partitions (no partition-broadcast op
    exists on DVE).
    Scores are computed transposed (S^T[k,q]) so softmax masking/exp output
    P^T feeds the AV matmul directly as the moving operand.  exp needs no
    max-subtraction: scores are O(1) bounded, masked entries use exact-0
    masks multiplied post-exp.
    Softmax normalization multiplies O^T by the replicated reciprocal rows
    during the PSUM->SBUF eviction; the head-concat X^T layout then feeds
    the output projection with Wo as the moving operand, and bo enters via
    a K=1 ones-row matmul.

The single fully-masked query row (q = S-1, uniform attention in the
reference) comes back NaN from the device and is recomputed exactly on the
host during the gather.
"""

import numpy as np
import ml_dtypes

B, S, D, H, A = 4, 2048, 512, 8, 64
P = 128
NQ = 1024          # q rows per core
NQT = 8            # q tiles per core
NKC = 16           # k chunks
NPAIR = 4          # head pairs
BF = ml_dtypes.bfloat16

# strip widths / offsets for the transposed-score layout
WKC = [P * (kc // 2 + 1) for kc in range(NKC)]
SOFF = np.concatenate([[0], np.cumsum(WKC)]).tolist()
PT_TOTAL = SOFF[-1]  # 9216

# exp groups: consecutive kc strips packed into <=1536-wide psum tiles
GROUPS = [(0, 6), (6, 8), (8, 10), (10, 12), (12, 13), (13, 14), (14, 15), (15, 16)]
STG_W = 1536

_cache = {}


def _split512(a, b):
    """Split [a,b) at multiples of 512 (PSUM bank boundaries)."""
    out = []
    while a < b:
        nxt = min(b, (a // 512 + 1) * 512)
        out.append((a, nxt))
        a = nxt
    return out


def _build():
    if "nc" in _cache:
        return _cache["nc"]

    import concourse.bacc as bacc
    import concourse.mybir as mybir
    import concourse.tile as tile

    F32 = mybir.dt.float32
    BF16 = mybir.dt.bfloat16
    MULT = mybir.AluOpType.mult
    EXP = mybir.ActivationFunctionType.Exp

    nc = bacc.Bacc("TRN2", target_bir_lowering=False, debug=False, num_devices=8)

    qT8_d = nc.dram_tensor("qT8", [P, 4 * NQ], BF16, kind="ExternalInput")
    kT_d = nc.dram_tensor("kT", [P, 4 * S], BF16, kind="ExternalInput")
    vT_d = nc.dram_tensor("vT", [P, 4 * S], BF16, kind="ExternalInput")
    wq_d = nc.dram_tensor("wq", [P, 2048], BF16, kind="ExternalInput")
    wk_d = nc.dram_tensor("wk", [P, 2048], BF16, kind="ExternalInput")
    wv_d = nc.dram_tensor("wv", [P, 2048], BF16, kind="ExternalInput")
    wo_d = nc.dram_tensor("wo", [P, 2048], BF16, kind="ExternalInput")
    bq_d = nc.dram_tensor("bq8", [P, 4], F32, kind="ExternalInput")
    bk_d = nc.dram_tensor("bk", [P, 4], F32, kind="ExternalInput")
    bv_d = nc.dram_tensor("bv", [P, 4], F32, kind="ExternalInput")
    bo_d = nc.dram_tensor("bo", [1, D], BF16, kind="ExternalInput")
    ones_d = nc.dram_tensor("ones1", [1, P], BF16, kind="ExternalInput")
    id_d = nc.dram_tensor("ident", [P, P], BF16, kind="ExternalInput")
    me_d = nc.dram_tensor("maskE", [P, P], BF16, kind="ExternalInput")
    mo_d = nc.dram_tensor("maskO", [P, P], BF16, kind="ExternalInput")
    out_d = nc.dram_tensor("out", [NQ, D], F32, kind="ExternalOutput")

    with tile.TileContext(nc) as tc:
        with (
            tc.tile_pool(name="cst", bufs=1) as cst,
            tc.tile_pool(name="act", bufs=1) as act,
            tc.tile_pool(name="vtt", bufs=2) as vtt,
            tc.tile_pool(name="ptg", bufs=9) as ptg,
            tc.tile_pool(name="rcp", bufs=3) as rcp,
            tc.tile_pool(name="ost", bufs=3) as ost,
            tc.tile_pool(name="psb", bufs=2, space="PSUM") as psb,
            tc.tile_pool(name="avp", bufs=2, space="PSUM") as avp,
        ):
            # ---- load everything ----
            qT8 = cst.tile([P, 4 * NQ], BF16, tag="qT8")
            kT = cst.tile([P, 4 * S], BF16, tag="kT")
            vT = cst.tile([P, 4 * S], BF16, tag="vT")
            wq = cst.tile([P, 2048], BF16, tag="wq")
            wk = cst.tile([P, 2048], BF16, tag="wk")
            wv = cst.tile([P, 2048], BF16, tag="wv")
            wo = cst.tile([P, 2048], BF16, tag="wo")
            bq = cst.tile([P, 4], F32, tag="bq")
            bk = cst.tile([P, 4], F32, tag="bk")
            bv = cst.tile([P, 4], F32, tag="bv")
            bo = cst.tile([1, D], BF16, tag="bo")
            on1 = cst.tile([1, P], BF16, tag="on1")
            idt = cst.tile([P, P], BF16, tag="idt")
            mE = cst.tile([P, P], BF16, tag="mE")
            mO = cst.tile([P, P], BF16, tag="mO")
            for t, d in [(qT8, qT8_d), (kT, kT_d), (vT, vT_d), (wq, wq_d),
                         (wk, wk_d), (wv, wv_d), (wo, wo_d), (bq, bq_d),
                         (bk, bk_d), (bv, bv_d), (bo, bo_d), (on1, ones_d),
                         (idt, id_d), (mE, me_d), (mO, mo_d)]:
                nc.sync.dma_start(t[:], d[:])

            QT = [act.tile([P, NQ], BF16, tag=f"QT{p}", name=f"QT{p}") for p in range(NPAIR)]
            KT = [act.tile([P, S], BF16, tag=f"KT{p}", name=f"KT{p}") for p in range(NPAIR)]
            Vn = [act.tile([P, NKC * 192], BF16, tag=f"Vn{p}", name=f"Vn{p}") for p in range(NPAIR)]
            XT = [act.tile([P, NQ], BF16, tag=f"XT{c}", name=f"XT{c}") for c in range(4)]

            # ---- projections for one head pair ----
            def project(p):
                for qh in range(NQ // 512):
                    ps = psb.tile([P, 512], F32, tag="big", padded_shape=[P, STG_W])
                    for ch in range(4):
                        nc.tensor.matmul(
                            ps[:], wq[:, (4 * p + ch) * P:(4 * p + ch + 1) * P],
                            qT8[:, NQ * ch + 512 * qh: NQ * ch + 512 * (qh + 1)],
                            start=(ch == 0), stop=(ch == 3))
                    nc.vector.tensor_scalar_add(
                        QT[p][:, 512 * qh:512 * (qh + 1)], ps[:], bq[:, p:p + 1])
                for sh in range(S // 512):
                    ps = psb.tile([P, 512], F32, tag="big", padded_shape=[P, STG_W])
                    for ch in range(4):
                        nc.tensor.matmul(
                            ps[:], wk[:, (4 * p + ch) * P:(4 * p + ch + 1) * P],
                            kT[:, S * ch + 512 * sh: S * ch + 512 * (sh + 1)],
                            start=(ch == 0), stop=(ch == 3))
                    nc.vector.tensor_scalar_add(
                        KT[p][:, 512 * sh:512 * (sh + 1)], ps[:], bk[:, p:p + 1])
                VTp = vtt.tile([P, S], BF16, tag="VT")
                for sh in range(S // 512):
                    ps = psb.tile([P, 512], F32, tag="big", padded_shape=[P, STG_W])
                    for ch in range(4):
                        nc.tensor.matmul(
                            ps[:], wv[:, (4 * p + ch) * P:(4 * p + ch + 1) * P],
                            vT[:, S * ch + 512 * sh: S * ch + 512 * (sh + 1)],
                            start=(ch == 0), stop=(ch == 3))
                    nc.vector.tensor_scalar_add(
                        VTp[:, 512 * sh:512 * (sh + 1)], ps[:], bv[:, p:p + 1])
                # V natural layout blocks [V_h0 | ones | V_h1] per k-chunk
                nc.gpsimd.memset(Vn[p][:], 1.0)
                for m in range(4):
                    vps = psb.tile([P, 512], BF16, tag="big", padded_shape=[P, 2 * STG_W])
                    for j in range(4):
                        nc.tensor.transpose(
                            vps[:, P * j:P * (j + 1)],
                            VTp[:, P * (4 * m + j):P * (4 * m + j + 1)], idt[:])
                    src = vps[:].rearrange("p (j c) -> p j c", c=P)
                    dst = Vn[p][:].rearrange("p (k c) -> p k c", c=192)
                    nc.vector.tensor_copy(dst[:, 4 * m:4 * m + 4, 0:64],
                                          src[:, :, 0:64])
                    nc.vector.tensor_copy(dst[:, 4 * m:4 * m + 4, 128:192],
                                          src[:, :, 64:128])

            # ---- attention for one head ----
            kc2g = {}
            for gi, (gs, ge) in enumerate(GROUPS):
                for kc in range(gs, ge):
                    kc2g[kc] = gi

            def attention(h):
                p, hh = h // 2, h % 2
                hr = slice(64 * hh, 64 * hh + 64)
                pts = []
                for (gs, ge) in GROUPS:
                    gw = SOFF[ge] - SOFF[gs]
                    stg = psb.tile([P, STG_W], F32, tag="big")
                    pt = ptg.tile([P, STG_W], BF16, tag="ptg")
                    for kc in range(gs, ge):
                        off = SOFF[kc] - SOFF[gs]
                        for (a0, a1) in _split512(off, off + WKC[kc]):
                            nc.tensor.matmul(
                                stg[:, a0:a1],
                                KT[p][hr, P * kc:P * (kc + 1)],
                                QT[p][hr, a0 - off:a1 - off],
                                start=True, stop=True)
                    nc.scalar.activation(pt[:, 0:gw], stg[:, 0:gw], EXP)
                    for kc in range(gs, ge):
                        off = SOFF[kc] - SOFF[gs]
                        nc.vector.tensor_tensor(
                            pt[:, off + WKC[kc] - P: off + WKC[kc]],
                            pt[:, off + WKC[kc] - P: off + WKC[kc]],
                            mE[:] if kc % 2 == 0 else mO[:], MULT)
                    pts.append(pt)
                # AV accumulation, one psum bank per 4 q-slots.  start=True only
                # on the bank's very first matmul: it clears has_written for the
                # whole bank, and later start=False matmuls overwrite-where-unset
                # (= first write) / accumulate-where-set, which is exactly the
                # per-slot accumulation semantics we need.
                orow = 0 if hh == 0 else 64
                drow = 64 - orow
                for b in range(2):
                    avb = avp.tile([P, 512], F32, tag="av")
                    kc0 = 8 * b
                    for kc in range(kc0, NKC):
                        gs = GROUPS[kc2g[kc]][0]
                        off = SOFF[kc] - SOFF[gs]
                        w = min(WKC[kc], 512 * (b + 1)) - 512 * b
                        nc.tensor.matmul(
                            avb[:, 0:w],
                            Vn[p][:, 192 * kc + 64 * hh: 192 * kc + 64 * hh + 128],
                            pts[kc2g[kc]][:, off + 512 * b: off + 512 * b + w],
                            start=(kc == kc0), stop=(kc == 15),
                            skip_group_check=True)
                    # approx-fast reciprocal is a custom DVE op that cannot read
                    # PSUM; bounce the replicated denominators through SBUF
                    rec = rcp.tile([64, 1024], F32, tag="rec")
                    nc.vector.tensor_copy(rec[:, 0:512], avb[drow:drow + 64, :])
                    nc.vector.reciprocal_approx_fast(rec[:, 512:1024], rec[:, 0:512])
                    nc.vector.tensor_tensor(XT[p][hr, 512 * b:512 * (b + 1)],
                                            avb[orow:orow + 64, :], rec[:, 512:1024],
                                            MULT)

            # interleave: each pair's heads start as soon as that pair's
            # projections are issued, so ACT/DVE warm up during projection
            for p in range(NPAIR):
                project(p)
                attention(2 * p)
                attention(2 * p + 1)

            # ---- output projection ----
            for i in range(NQT):
                po = psb.tile([P, D], F32, tag="big", padded_shape=[P, STG_W])
                for ch in range(4):
                    nc.tensor.matmul(po[:], XT[ch][:, P * i:P * (i + 1)],
                                     wo[:, 512 * ch:512 * (ch + 1)],
                                     start=(ch == 0), stop=False)
                nc.tensor.matmul(po[:], on1[0:1, :], bo[0:1, :],
                                 start=False, stop=True)
                ob = ost.tile([P, D], F32, tag="ob")
                nc.vector.tensor_copy(ob[:], po[:])
                nc.sync.dma_start(out_d[P * i:P * (i + 1), :], ob[:])

    nc.compile()
    _cache["nc"] = nc
    return nc


def _host_prep(query, key, value, Wq, bq, Wk, bk, Wv, bv, Wo, bo):
    """Build the 8 per-core input maps (all device-side layouts)."""
    def stack_pairs(W):
        # [H,D,A] -> [128, 16*128]: col block (4p+ch) = rows 128ch of [Wq_2p|Wq_2p+1]
        blocks = []
        for p in range(NPAIR):
            Wp = np.concatenate([W[2 * p], W[2 * p + 1]], axis=1)  # [512, 128]
            for ch in range(4):
                blocks.append(Wp[P * ch:P * (ch + 1), :])
        return np.stack(blocks, 1).reshape(P, -1).astype(BF)  # [128, 16, 128]->[128,2048]

    wq_h, wk_h, wv_h = stack_pairs(Wq), stack_pairs(Wk), stack_pairs(Wv)
    wo_h = np.stack([Wo[P * ch:P * (ch + 1), :] for ch in range(4)], 1)
    wo_h = wo_h.reshape(P, -1).astype(BF)  # [128, 4*512]

    def stack_bias(b, scale=1.0):
        cols = [np.concatenate([b[2 * p], b[2 * p + 1]]) * scale for p in range(NPAIR)]
        return np.stack(cols, 1).astype(np.float32)  # [128, 4]

    bq_h = stack_bias(bq, 0.125)
    bk_h, bv_h = stack_bias(bk), stack_bias(bv)
    bo_h = bo[None, :].astype(BF)
    ones_h = np.ones((1, P), BF)
    id_h = np.eye(P, dtype=BF)
    kl = np.arange(P)[:, None]
    ql = np.arange(P)[None, :]
    tril_strict = (kl > ql).astype(BF)

    def chunked_T(x, scale=1.0):
        # [S', D] -> [128, 4*S'] with col block ch = rows 128ch of x.T
        xT = np.ascontiguousarray(x.T) * scale  # [512, S']
        return xT.reshape(4, P, -1).transpose(1, 0, 2).reshape(P, -1).astype(BF)

    in_maps = []
    for c in range(8):
        b, pair = c // 2, c % 2
        sel = np.concatenate(
            [np.arange(P * (2 * i + pair), P * (2 * i + pair) + P) for i in range(NQT)])
        m = {
            "qT8": chunked_T(query[b][sel], 0.125),
            "kT": chunked_T(key[b]),
            "vT": chunked_T(value[b]),
            "wq": wq_h, "wk": wk_h, "wv": wv_h, "wo": wo_h,
            "bq8": bq_h, "bk": bk_h, "bv": bv_h, "bo": bo_h,
            "ones1": ones_h, "ident": id_h,
            "maskE": tril_strict if pair == 0 else np.zeros((P, P), BF),
            "maskO": np.ones((P, P), BF) if pair == 0 else tril_strict,
        }
        in_maps.append(m)
    return in_maps


def kernel(query, key, value, Wq, bq, Wk, bk, Wv, bv, Wo, bo):
    from concourse.bass_utils import run_bass_kernel_spmd

    args = [np.asarray(a, dtype=np.float32) for a in
            (query, key, value, Wq, bq, Wk, bk, Wv, bv, Wo, bo)]
    query, key, value, Wq, bq, Wk, bk, Wv, bv, Wo, bo = args

    nc = _build()
    in_maps = _host_prep(*args)
    res = run_bass_kernel_spmd(nc, in_maps, list(range(8)))

    out = np.empty((B, S, D), np.float32)
    for c in range(8):
        b, pair = c // 2, c % 2
        o = res.results[c]["out"]
        for i in range(NQT):
            g = 2 * i + pair
            out[b, P * g:P * (g + 1), :] = o[P * i:P * (i + 1), :]

    # q = S-1 attends to nothing -> reference softmax is uniform over all keys
    for b in range(B):
        vm = value[b].mean(0)
        x = np.concatenate([vm @ Wv[h] + bv[h] for h in range(H)])
        out[b, S - 1, :] = x @ Wo + bo
    return out



# revision 5
# speedup vs baseline: 1.4442x; 1.4442x over previous
"""Multi-head attention (strictly-future mask) on 8 TRN2 cores — v2.

Reference math (B=4, S=2048, D=512, H=8, A=64):
    q/k/v = per-head projections                              [B,H,S,A]
    scores = q @ k^T / 8, lower triangle (k <= q) masked to -1e9
    out = concat_heads(softmax(scores) @ v) @ Wo + bo         [B,S,D]

Sharding: head-parallel within a batch — core c = (batch b = c//2,
head-half hp = c%2).  Each core computes 4 heads (= 2 stacked head
pairs) over the FULL 2048-query range, producing a partial output
summed on the host (Wo split along its input axis, per the TP hint);
host reduction replaces the all-reduce.

Bias algebra: softmax cancels any per-query additive score term, so
bk is dropped (its q-dependent term cancels) and only bq is kept on Q
(it produces the surviving per-key term).  bv contributes exactly
bv @ Wo_head (softmax weights sum to 1) — folded with bo into a host
constant.  K/V psum evictions are therefore pure copies.

Device dataflow (per core):
  * Q/K projections (transposed [a,S] layout) and V projection
    (natural [k,a] layout, input as stationary operand — no PE
    transposes needed) run as fp8e4 DoubleRow matmuls: contraction
    512 = 2x(2x128) chunk pairs at 0.5 cycles/column.
  * Scores are computed transposed (S^T[k,q]) in bf16; per head-pair
    strips kc carry a uniform pair width w_j = 256(j+1) so DoubleRow
    can pair even/odd k-strips in the AV stage.  exp() on the scalar
    engine writes P^T directly as fp8e4; diagonal blocks are masked
    post-exp by a 0/1 tril multiply; the even strip's overhang block
    is never computed — just zero-filled.
  * AV runs as fp8 DoubleRow over strip pairs with the [V|ones|V]
    stationary trick replicating softmax denominators; normalization
    divides via copy + reciprocal_approx_fast on DVE.
  * Output projection consumes X^T per 128-query tile, producing the
    natural [2048, 512] f32 partial output.

The last 32 query rows (tiny attention fan-in, where fp8 quantization
noise is largest — and q = S-1 is 0/0) are recomputed exactly on the
host during the gather.
"""

import numpy as np
import ml_dtypes

B, S, D, H, A = 4, 2048, 512, 8, 64
P = 128
NPAIR = 2            # head pairs per core
NHEAD = 4            # heads per core
NJ = 8               # strip pairs per head
FIX_ROWS = 32        # host-recomputed tail rows
BF = ml_dtypes.bfloat16
F8 = ml_dtypes.float8_e4m3

W_J = [256 * (j + 1) for j in range(NJ)]   # uniform pair strip widths

_cache = {}


def _chunks(lo, hi, step):
    out = []
    while lo < hi:
        out.append((lo, min(hi, lo + step)))
        lo = out[-1][1]
    return out


def _build():
    if "nc" in _cache:
        return _cache["nc"]

    import concourse.bacc as bacc
    import concourse.mybir as mybir
    import concourse.tile as tile

    F32 = mybir.dt.float32
    BF16 = mybir.dt.bfloat16
    FP8 = mybir.dt.float8e4
    MULT = mybir.AluOpType.mult
    ADD = mybir.AluOpType.add
    EXP = mybir.ActivationFunctionType.Exp
    DR = mybir.MatmulPerfMode.DoubleRow

    nc = bacc.Bacc("TRN2", target_bir_lowering=False, debug=False, num_devices=8)

    xq_d = nc.dram_tensor("xq", [P, 4 * S], FP8, kind="ExternalInput")
    xk_d = nc.dram_tensor("xk", [P, 4 * S], FP8, kind="ExternalInput")
    xv_d = nc.dram_tensor("xv", [P, 4 * S], FP8, kind="ExternalInput")
    wq_d = nc.dram_tensor("wq", [P, 1024], FP8, kind="ExternalInput")
    wk_d = nc.dram_tensor("wk", [P, 1024], FP8, kind="ExternalInput")
    wv_d = nc.dram_tensor("wv", [P, 1024], FP8, kind="ExternalInput")
    wo_d = nc.dram_tensor("wo", [P, 1024], BF16, kind="ExternalInput")
    bq_d = nc.dram_tensor("bq", [P, 2], F32, kind="ExternalInput")
    mk_d = nc.dram_tensor("mask", [P, P], BF16, kind="ExternalInput")
    out_d = nc.dram_tensor("out", [S, D], F32, kind="ExternalOutput")

    with tile.TileContext(nc) as tc:
        with (
            tc.tile_pool(name="cst", bufs=1) as cst,
            tc.tile_pool(name="act", bufs=1) as act,
            tc.tile_pool(name="pts", bufs=2) as pts,
            tc.tile_pool(name="rcp", bufs=2) as rcp,
            tc.tile_pool(name="ost", bufs=3) as ost,
            tc.tile_pool(name="stg", bufs=2, space="PSUM") as stg,
            tc.tile_pool(name="avp", bufs=4, space="PSUM") as avp,
        ):
            xq = cst.tile([P, 4 * S], FP8, tag="xq")
            xk = cst.tile([P, 4 * S], FP8, tag="xk")
            xv = cst.tile([P, 4 * S], FP8, tag="xv")
            wq = cst.tile([P, 1024], FP8, tag="wq")
            wk = cst.tile([P, 1024], FP8, tag="wk")
            wv = cst.tile([P, 1024], FP8, tag="wv")
            wo = cst.tile([P, 1024], BF16, tag="wo")
            bq = cst.tile([P, 2], F32, tag="bq")
            mk = cst.tile([P, P], BF16, tag="mk")

            # load order: Q-proj deps first so compute starts early
            nc.sync.dma_start(wq[:], wq_d[:])
            nc.sync.dma_start(bq[:], bq_d[:])
            nc.sync.dma_start(xq[:, 0:4096], xq_d[:, 0:4096])
            nc.sync.dma_start(xq[:, 4096:8192], xq_d[:, 4096:8192])
            nc.sync.dma_start(wk[:], wk_d[:])
            nc.sync.dma_start(xk[:, 0:4096], xk_d[:, 0:4096])
            nc.sync.dma_start(xk[:, 4096:8192], xk_d[:, 4096:8192])
            nc.sync.dma_start(mk[:], mk_d[:])
            nc.sync.dma_start(wv[:], wv_d[:])
            nc.sync.dma_start(xv[:, 0:4096], xv_d[:, 0:4096])
            nc.sync.dma_start(xv[:, 4096:8192], xv_d[:, 4096:8192])
            nc.sync.dma_start(wo[:], wo_d[:])

            QT = [act.tile([P, S], BF16, tag=f"QT{p}", name=f"QT{p}") for p in range(NPAIR)]
            KT = [act.tile([P, S], BF16, tag=f"KT{p}", name=f"KT{p}") for p in range(NPAIR)]
            Vn = [act.tile([P, 16 * 192], FP8, tag=f"Vn{p}", name=f"Vn{p}") for p in range(NPAIR)]
            XT = [act.tile([P, S], BF16, tag=f"XT{p}", name=f"XT{p}") for p in range(NPAIR)]

            xq4 = xq[:].rearrange("p (ch c) -> p ch c", ch=4)
            xk4 = xk[:].rearrange("p (ch c) -> p ch c", ch=4)
            xv4 = xv[:].rearrange("p (ch c) -> p ch c", ch=4)

            def proj_qk(p):
                # Q then K: transposed layout via fp8 DoubleRow, 1024-col psum groups
                for src4, wt, dstT, is_q in ((xq4, wq, QT[p], True),
                                             (xk4, wk, KT[p], False)):
                    for g in range(2):
                        ps = stg.tile([P, 1024], F32, tag="stg")
                        for half in range(2):
                            q0 = 1024 * g + 512 * half
                            for cp in range(2):
                                wview = wt[:, (p * 2 + cp) * 256:(p * 2 + cp + 1) * 256]
                                nc.tensor.matmul(
                                    ps[:, 512 * half:512 * (half + 1)],
                                    wview.rearrange("p (two c) -> p two c", two=2),
                                    src4[:, 2 * cp:2 * cp + 2, q0:q0 + 512],
                                    start=(cp == 0), stop=(cp == 1),
                                    perf_mode=DR)
                        if is_q:
                            nc.vector.tensor_scalar(
                                dstT[:, 1024 * g:1024 * (g + 1)], ps[:],
                                0.125, bq[:, p:p + 1], MULT, ADD)
                        else:
                            nc.scalar.copy(dstT[:, 1024 * g:1024 * (g + 1)], ps[:])

            def proj_v(p):
                nc.gpsimd.memset(Vn[p][:], 1.0)
                for g in range(4):  # 4 k-blocks per psum tile
                    ps = avp.tile([P, 512], F32, tag="av")
                    first = True
                    for kb in range(4):
                        for cp in range(2):
                            wview = wv[:, (p * 2 + cp) * 256:(p * 2 + cp + 1) * 256]
                            nc.tensor.matmul(
                                ps[:, P * kb:P * (kb + 1)],
                                xv4[:, 2 * cp:2 * cp + 2,
                                    P * (4 * g + kb):P * (4 * g + kb + 1)],
                                wview.rearrange("p (two c) -> p two c", two=2),
                                start=first, stop=(kb == 3 and cp == 1),
                                perf_mode=DR, skip_group_check=True)
                            first = False
                    # evict into the [V_h0|ones|V_h1] per-chunk pattern
                    src = ps[:].rearrange("p (kb hh c) -> p kb hh c", kb=4, c=64)
                    dst = Vn[p][:].rearrange("p (kc t c) -> p kc t c", kc=16, c=64)
                    for hh in range(2):
                        nc.vector.tensor_copy(
                            dst[:, 4 * g:4 * g + 4, 2 * hh:2 * hh + 1, :],
                            src[:, :, hh:hh + 1, :])

            def scores(h):
                p, hh = h // 2, h % 2
                hr = slice(64 * hh, 64 * hh + 64)
                tiles = []
                for j in range(NJ):
                    w = W_J[j]
                    pt = pts.tile([P, 2 * w], FP8, tag=f"pt{j}")
                    tiles.append(pt)
                    # segments: even strip [0, w-128) valid, odd strip [w, 2w)
                    for kc, sbase, slen in ((2 * j, 0, w - 128), (2 * j + 1, w, w)):
                        for c0, c1 in _chunks(0, slen, 1024):
                            ps = stg.tile([P, 1024], F32, tag="stg")
                            for a0, a1 in _chunks(c0, c1, 512):
                                nc.tensor.matmul(
                                    ps[:, a0 - c0:a1 - c0],
                                    KT[p][hr, P * kc:P * (kc + 1)],
                                    QT[p][hr, a0:a1],
                                    start=True, stop=True)
                            nc.scalar.activation(
                                pt[:, sbase + c0:sbase + c1],
                                ps[:, 0:c1 - c0], EXP)
                    # masks: even diag block, odd diag block; zero the overhang
                    nc.vector.memset(pt[:, w - 128:w], 0.0)
                    nc.vector.tensor_tensor(
                        pt[:, w - 256:w - 128], pt[:, w - 256:w - 128], mk[:], MULT)
                    nc.vector.tensor_tensor(
                        pt[:, 2 * w - 128:2 * w], pt[:, 2 * w - 128:2 * w], mk[:], MULT)
                return tiles

            def av(h, tiles):
                p, hh = h // 2, h % 2
                hr = slice(64 * hh, 64 * hh + 64)
                orow, drow = (0, 64) if hh == 0 else (64, 0)
                for b in range(4):
                    avb = avp.tile([P, 512], F32, tag="av")
                    for j in range(2 * b, NJ):
                        w = W_J[j]
                        ln = min(w, 512 * (b + 1)) - 512 * b
                        pt2 = tiles[j][:].rearrange("p (two w) -> p two w", two=2)
                        vv = Vn[p][:].rearrange("p (kc c) -> p kc c", c=192)
                        nc.tensor.matmul(
                            avb[:, 0:ln],
                            vv[:, 2 * j:2 * j + 2, 64 * hh:64 * hh + 128],
                            pt2[:, :, 512 * b:512 * b + ln],
                            start=(j == 2 * b), stop=(j == NJ - 1),
                            perf_mode=DR, skip_group_check=True)
                    rec = rcp.tile([64, 1024], F32, tag="rec")
                    nc.vector.tensor_copy(rec[:, 0:512], avb[drow:drow + 64, :])
                    nc.vector.reciprocal_approx_fast(rec[:, 512:1024], rec[:, 0:512])
                    nc.vector.tensor_tensor(
                        XT[p][hr, 512 * b:512 * (b + 1)],
                        avb[orow:orow + 64, :], rec[:, 512:1024], MULT)

            proj_qk(0)
            t0 = scores(0)
            proj_qk(1)
            proj_v(0)
            t1 = scores(1)
            av(0, t0)
            proj_v(1)
            t2 = scores(2)
            av(1, t1)
            t3 = scores(3)
            av(2, t2)
            av(3, t3)

            # output projection: natural [q, d] partial result
            for i in range(16):
                po = avp.tile([P, 512], F32, tag="av")
                for p in range(NPAIR):
                    nc.tensor.matmul(po[:], XT[p][:, P * i:P * (i + 1)],
                                     wo[:, 512 * p:512 * (p + 1)],
                                     start=(p == 0), stop=(p == NPAIR - 1))
                ob = ost.tile([P, 512], F32, tag="ob")
                if i % 2 == 0:
                    nc.vector.tensor_copy(ob[:], po[:])
                else:
                    nc.scalar.copy(ob[:], po[:])
                nc.sync.dma_start(out_d[P * i:P * (i + 1), :], ob[:])

    nc.compile()
    _cache["nc"] = nc
    return nc


def _host_prep(query, key, value, Wq, bq, Wk, bk, Wv, bv, Wo, bo):
    """Build the 8 per-core input maps."""
    def chunked_T(x):
        # [S, D] -> [128, 4*S]: col block ch = rows 128ch of x.T
        xT = np.ascontiguousarray(x.T)  # [512, S]
        return xT.reshape(4, P, S).transpose(1, 0, 2).reshape(P, 4 * S)

    xt = {b: {"q": chunked_T(query[b]).astype(F8),
              "k": chunked_T(key[b]).astype(F8),
              "v": chunked_T(value[b]).astype(F8)} for b in range(B)}

    kl = np.arange(P)[:, None]
    ql = np.arange(P)[None, :]
    mask = (kl > ql).astype(BF)

    in_maps = []
    for c in range(8):
        b, hp = c // 2, c % 2
        heads = range(4 * hp, 4 * hp + 4)

        def stat_pack(W):
            # stationary DR layout: block (p, cp, t) = W2[128*(2cp+t)] rows
            blocks = []
            for p in range(NPAIR):
                hg = 4 * hp + 2 * p
                W2 = np.concatenate([W[hg], W[hg + 1]], axis=1)  # [512, 128]
                for cp in range(2):
                    for t in range(2):
                        blocks.append(W2[P * (2 * cp + t):P * (2 * cp + t + 1), :])
            return np.concatenate(blocks, axis=1).astype(F8)  # [128, 1024]

        bq_h = np.stack(
            [np.concatenate([bq[4 * hp + 2 * p], bq[4 * hp + 2 * p + 1]])
             for p in range(NPAIR)], axis=1).astype(np.float32) / 8.0

        wo_h = np.concatenate(
            [Wo[64 * (4 * hp + 2 * p):64 * (4 * hp + 2 * p + 2), :]
             for p in range(NPAIR)], axis=1).astype(BF)  # [128, 1024]

        in_maps.append({
            "xq": xt[b]["q"], "xk": xt[b]["k"], "xv": xt[b]["v"],
            "wq": stat_pack(Wq), "wk": stat_pack(Wk), "wv": stat_pack(Wv),
            "wo": wo_h, "bq": bq_h, "mask": mask,
        })
    return in_maps


def kernel(query, key, value, Wq, bq, Wk, bk, Wv, bv, Wo, bo):
    from concourse.bass_utils import run_bass_kernel_spmd

    args = [np.asarray(a, dtype=np.float32) for a in
            (query, key, value, Wq, bq, Wk, bk, Wv, bv, Wo, bo)]
    query, key, value, Wq, bq, Wk, bk, Wv, bv, Wo, bo = args

    nc = _build()
    in_maps = _host_prep(*args)
    res = run_bass_kernel_spmd(nc, in_maps, list(range(8)))

    # host gather: sum the two head-half partials + bias constant
    const = (bo + bv.reshape(-1) @ Wo).astype(np.float32)  # bv via softmax-sums-to-1
    out = np.empty((B, S, D), np.float32)
    for b in range(B):
        out[b] = res.results[2 * b]["out"] + res.results[2 * b + 1]["out"] + const

    # exact host recompute of the last FIX_ROWS rows (tiny fan-in + q=S-1)
    scale = 1.0 / np.sqrt(A)
    for b in range(B):
        vm = value[b].mean(0)
        x = np.concatenate([vm @ Wv[h] + bv[h] for h in range(H)])
        out[b, S - 1, :] = x @ Wo + bo
        for q in range(S - FIX_ROWS, S - 1):
            ks = np.arange(q + 1, S)
            xrow = []
            for h in range(H):
                qh = query[b, q] @ Wq[h] + bq[h]
                kh = key[b, ks] @ Wk[h] + bk[h]
                vh = value[b, ks] @ Wv[h] + bv[h]
                sc = (kh @ qh) * scale
                sc -= sc.max()
                pw = np.exp(sc)
                pw /= pw.sum()
                xrow.append(pw @ vh)
            out[b, q, :] = np.concatenate(xrow) @ Wo + bo
    return out
